# revision 12
# baseline (speedup 1.0000x reference)
"""ChebyKAN layer kernel for 8x Trainium2 NeuronCores.

Computes y[b,o] = sum_{i,d} T_d(tanh(x[b,i])) * C[i,o,d], d = 0..8, via:
  - batch sharded 8 ways (1024 rows/core)
  - fp32 Chebyshev recurrence for T_1..T_8 on ACT/DVE
  - d=0 term (T_0 == 1) folded into a host-precomputed bias[o]
  - HYBRID contraction, K = (i,d) of size 8192:
      degrees {1,2,3} (24 K-chunks) as fp8(e4m3) DoubleRow matmuls --
      2 K-chunks per instruction at 2x throughput;
      degrees {4..8} (40 K-chunks) as bf16 matmuls.
    All weights are pre-scaled by 2^14 (exact power of two) so e4m3
    holds them with full mantissa precision and BOTH dtypes accumulate
    at the same scale into the same PSUM bank; the 2^-14 is folded into
    the bias-add at PSUM drain. Max rel err vs fp32 reference ~1.7e-2
    (tolerance 2e-2).
  - weights stream as 4-chunk "supertiles" (one DMA descriptor per 4
    K-chunks) so the DMA issue rate never starves the PE
  - x is transposed on host so the basis is produced directly in
    [K, batch] (lhsT) layout; no on-device transpose needed.

Self-contained: hardcodes all shapes for inputs
  x: [8192, 1024] f32, cheby_coeffs: [1024, 1024, 9] f32.
"""

import numpy as np
import ml_dtypes

import concourse.bass as bass
import concourse.mybir as mybir
import concourse.tile as tile
from concourse import bacc
from concourse.bass_utils import run_bass_kernel_spmd

P = 128
B_TOTAL = 8192
I_DIM = 1024
O_DIM = 1024
N_CORES = 8
B_LOCAL = B_TOTAL // N_CORES     # 1024
IC = I_DIM // P                  # 8 input chunks
OH = 2                           # output halves (PSUM bank = 512 fp32)
ON = O_DIM // OH                 # 512

NB16 = 5                         # bf16 degrees 4..8
NK16 = IC * NB16                 # 40 bf16 K-chunks
NST16 = NK16 // 4                # 10 bf16 supertiles (4 chunks each)
NP8 = IC + IC // 2               # 12 fp8 pairs: A[ic]=(d1,d2), B[j]=(d3,d3)
NST8 = NP8 // 4                  # 3 fp8 supertiles (4 pairs each)
WSCALE = 2.0 ** 14               # weight pre-scale (exact, shared by dtypes)
WSINV = 2.0 ** -14

_nc = None
last_results = None  # BassKernelResults of the most recent run (for profiling)


def _ensure_ntff_hook():
    """bass_utils' trace path imports antenv.axon_hooks unconditionally, but
    this agent image's antenv package lacks that module. Synthesize it (with
    the real libaxon NTFF hook when available) so a BASS_TRACE=1 run traces
    instead of crashing."""
    import sys
    import types

    try:
        import antenv.axon_hooks  # noqa: F401
        return
    except ImportError:
        pass
    try:
        import antenv
    except ImportError:
        return
    hook = None
    try:
        from trn_agent_boot.trn_boot import _ntff_profile_via_ctypes
        hook = _ntff_profile_via_ctypes("/opt/axon/libaxon_pjrt.so")
    except Exception:
        hook = None
    mod = types.ModuleType("antenv.axon_hooks")
    state = {"hook": hook}
    mod.set_axon_ntff_profile_hook = lambda h: state.__setitem__("hook", h)
    mod.get_axon_ntff_profile_hook = lambda: state["hook"]
    sys.modules["antenv.axon_hooks"] = mod
    antenv.axon_hooks = mod


_ensure_ntff_hook()


def _chunk_order():
    """Per-o-half K-chunk schedule: ("b", k16) bf16 chunks and ("p", pid)
    fp8 pairs, ordered so every chunk's basis is ready shortly after its
    ic's recurrence runs. Pair A[ic]=(T1,T2) leads each ic (earliest
    ready); pair B[j]=(T3@2j, T3@2j+1) follows ic=2j+1."""
    order = []
    for ic in range(IC):
        order.append(("p", ic))
        for di in range(NB16):
            order.append(("b", ic * NB16 + di))
        if ic % 2 == 1:
            order.append(("p", IC + ic // 2))
    return order


def _build_nc():
    nc = bacc.Bacc()
    f32 = mybir.dt.float32
    bf16 = mybir.dt.bfloat16
    f8 = mybir.dt.float8e4
    AF = mybir.ActivationFunctionType
    ALU = mybir.AluOpType
    DR = mybir.MatmulPerfMode.DoubleRow

    xt_d = nc.dram_tensor("xt", [I_DIM, B_LOCAL], f32, kind="ExternalInput")
    w16_d = nc.dram_tensor("w16", [OH, NST16, P, 4, ON], bf16,
                           kind="ExternalInput")
    w8_d = nc.dram_tensor("w8", [OH, NST8, P, 4, 2, ON], f8,
                          kind="ExternalInput")
    bias_d = nc.dram_tensor("bias", [P, O_DIM], f32, kind="ExternalInput")
    y_d = nc.dram_tensor("y", [B_LOCAL, O_DIM], f32, kind="ExternalOutput")

    order = _chunk_order()
    nchunk = len(order)

    with tile.TileContext(nc) as tc:
        with (
            tc.tile_pool(name="const", bufs=1) as cpool,
            tc.tile_pool(name="xin", bufs=2) as xpool,
            tc.tile_pool(name="fwork", bufs=2) as fpool,
            tc.tile_pool(name="basis", bufs=1) as bpool,
            tc.tile_pool(name="w16s", bufs=4) as wpool,
            tc.tile_pool(name="w8s", bufs=2) as w8pool,
            tc.tile_pool(name="outbuf", bufs=4) as opool,
            tc.tile_pool(name="acc", bufs=1, space="PSUM") as ppool,
        ):
            # ---- first xt half + per-chunk prefetch of the first weight
            # supertiles. The sync queue's wire rate (~140 GB/s) makes DMA
            # ORDER matter: the basis chain is gated on xt0a, and the first
            # matmuls only need chunk 0 of each supertile, so those 128KB
            # pieces go ahead of everything bulky.
            st16 = {}
            st8 = {}
            xt0 = xpool.tile([P, B_LOCAL], f32, tag="xt", name="xt_0")
            nc.sync.dma_start(out=xt0, in_=xt_d[0:P, :])
            st8[(0, 0)] = w8pool.tile([P, 4, 2, ON], f8, tag="wt8",
                                      name="wt8_p0")
            nc.sync.dma_start(out=st8[(0, 0)][:, 0, :, :], in_=w8_d[0, 0, :, 0])
            st16[(0, 0)] = wpool.tile([P, 4, ON], bf16, tag="wt16",
                                      name="wt16_p0")
            nc.sync.dma_start(out=st16[(0, 0)][:, 0, :], in_=w16_d[0, 0, :, 0])
            for c in range(1, 4):
                nc.sync.dma_start(out=st16[(0, 0)][:, c, :],
                                  in_=w16_d[0, 0, :, c])
            for c in range(1, 4):
                nc.sync.dma_start(out=st8[(0, 0)][:, c, :, :],
                                  in_=w8_d[0, 0, :, c])

            # ---- PE warm-up ----
            # HAM un-throttles the PE clock (1.2 -> 2.4 GHz) only after
            # ~3.4us of sustained matmul activity; bridge until the first
            # basis chunk is ready (~12us) with dummy N=512 matmuls. memset
            # on gpsimd: its queue clears the trace barrier earliest.
            warm = cpool.tile([P, ON], bf16, name="warm")
            nc.gpsimd.memset(warm, 1.0)
            warm_ps = ppool.tile([P, ON], f32, tag="ps0", name="warm_ps")
            for wi in range(8):
                nc.tensor.matmul(warm_ps, warm[:, 0:P], warm,
                                 start=(wi == 0), stop=(wi == 7))

            # ---- basis production per 128-row chunk of i ----
            # pairs A[ic]: [128, 2, B] e4m3, slots (T1, T2) of ic
            # pairs B[j]:  [128, 2, B] e4m3, slots (T3@ic=2j, T3@ic=2j+1)
            # bf16[ic]:    [128, 5, B] bf16, slots (T4..T8) of ic
            # one tag per family with bufs = #tiles: same memory, far fewer
            # semaphores (the end-of-kernel semaphore-reset epilogue is
            # serialized per distinct semaphore).
            pairs = {}
            for ic in range(IC):
                pairs[ic] = bpool.tile([P, 2, B_LOCAL], f8, tag="pA",
                                       bufs=IC, name=f"pA{ic}")
            for j in range(IC // 2):
                pairs[IC + j] = bpool.tile([P, 2, B_LOCAL], f8, tag="pB",
                                           bufs=IC // 2, name=f"pB{j}")
            b16 = {}
            for ic in range(IC):
                b16[ic] = bpool.tile([P, NB16, B_LOCAL], bf16, tag="b16",
                                     bufs=IC, name=f"b{ic}")

            for ic in range(IC):
                # ic == 0 runs every op on two half-tiles so the first
                # K-chunks are ready earlier (overlaps PE ramp-up).
                slices = ([slice(0, B_LOCAL // 2), slice(B_LOCAL // 2, B_LOCAL)]
                          if ic == 0 else [slice(0, B_LOCAL)])

                if ic == 0:
                    xt_t = xt0  # prefetched above
                else:
                    xt_t = xpool.tile([P, B_LOCAL], f32, tag="xt",
                                      name=f"xt_{ic}")
                    # full-tile DMA (4KB/partition contiguous beats two 2KB
                    # halves on wire efficiency); alternate queues so neither
                    # queue's wire time gates the basis chain
                    q = nc.gpsimd if ic % 2 == 1 else nc.sync
                    q.dma_start(out=xt_t, in_=xt_d[ic * P:(ic + 1) * P, :])

                pA = pairs[ic]
                pB = pairs[IC + ic // 2]
                bb = b16[ic]

                t = fpool.tile([P, B_LOCAL], f32, tag="T1", name=f"t_{ic}")
                s2 = fpool.tile([P, B_LOCAL], f32, tag="sq", name=f"s2_{ic}")
                T2 = fpool.tile([P, B_LOCAL], f32, tag="Tn", name=f"T2_{ic}",
                                bufs=3)
                V3 = fpool.tile([P, B_LOCAL], f32, tag="u", name=f"V3_{ic}")
                T3 = fpool.tile([P, B_LOCAL], f32, tag="Tn", name=f"T3_{ic}",
                                bufs=3)
                s4 = fpool.tile([P, B_LOCAL], f32, tag="sq", name=f"s4_{ic}")
                T4 = fpool.tile([P, B_LOCAL], f32, tag="Tn", name=f"T4_{ic}",
                                bufs=3)
                u5 = fpool.tile([P, B_LOCAL], f32, tag="u", name=f"u5_{ic}")
                s6 = fpool.tile([P, B_LOCAL], f32, tag="sq", name=f"s6_{ic}")
                u7 = fpool.tile([P, B_LOCAL], f32, tag="u", name=f"u7_{ic}")
                s8 = fpool.tile([P, B_LOCAL], f32, tag="sq", name=f"s8_{ic}")

                for sl in slices:
                    # T1 = tanh(x) (no clip: recurrence stable for |t|<=1;
                    # deviation from the reference's 0.999 clip is ~1e-6)
                    nc.scalar.activation(t[:, sl], xt_t[:, sl], AF.Tanh)
                    nc.vector.tensor_copy(pA[:, 0, sl], t[:, sl])

                    # T2 = 2 t^2 - 1
                    nc.scalar.square(s2[:, sl], t[:, sl])
                    nc.vector.tensor_scalar(T2[:, sl], s2[:, sl], 2.0, -1.0,
                                            ALU.mult, ALU.add)
                    nc.scalar.copy(pA[:, 1, sl], T2[:, sl])

                    # T3 = t * (2 T2 - 1)
                    nc.vector.tensor_scalar(V3[:, sl], T2[:, sl], 2.0, -1.0,
                                            ALU.mult, ALU.add)
                    nc.vector.tensor_mul(T3[:, sl], t[:, sl], V3[:, sl])
                    nc.scalar.copy(pB[:, ic % 2, sl], T3[:, sl])

                    # T4 = 2 T2^2 - 1
                    nc.scalar.square(s4[:, sl], T2[:, sl])
                    nc.vector.tensor_scalar(T4[:, sl], s4[:, sl], 2.0, -1.0,
                                            ALU.mult, ALU.add)
                    nc.scalar.copy(bb[:, 0, sl], T4[:, sl])

                    # T5 = 2 T2 T3 - T1, T7 = 2 T3 T4 - T1 (fp32 products so
                    # the bf16 basis keeps full bf16 accuracy)
                    nc.vector.tensor_mul(u5[:, sl], T2[:, sl], T3[:, sl])
                    nc.vector.scalar_tensor_tensor(bb[:, 1, sl], u5[:, sl],
                                                   2.0, t[:, sl],
                                                   ALU.mult, ALU.subtract)

                    # T6 = 2 T3^2 - 1, T8 = 2 T4^2 - 1
                    nc.scalar.square(s6[:, sl], T3[:, sl])
                    nc.vector.tensor_scalar(bb[:, 2, sl], s6[:, sl], 2.0, -1.0,
                                            ALU.mult, ALU.add)

                    nc.vector.tensor_mul(u7[:, sl], T3[:, sl], T4[:, sl])
                    nc.vector.scalar_tensor_tensor(bb[:, 3, sl], u7[:, sl],
                                                   2.0, t[:, sl],
                                                   ALU.mult, ALU.subtract)

                    nc.scalar.square(s8[:, sl], T4[:, sl])
                    nc.vector.tensor_scalar(bb[:, 4, sl], s8[:, sl], 2.0, -1.0,
                                            ALU.mult, ALU.add)

            # bias is only consumed at the end of each pass; load it late so
            # it doesn't delay the xt/wt streams.
            bias_t = cpool.tile([P, O_DIM], f32, name="bias_t")
            nc.sync.dma_start(out=bias_t, in_=bias_d[:, :])

            # ---- contraction passes ----
            psums = [ppool.tile([P, ON], f32, tag=f"ps{b}", name=f"ps{b}")
                     for b in range(B_LOCAL // P)]
            # pass 0: o-half 0, all 8 batch banks (overlaps basis production)
            # pass 1a/1b: o-half 1 split in two bank halves, so the first
            # half's drains + stores overlap the second half's matmuls.
            passes = [(0, 0, 8), (1, 0, 4), (1, 4, 8)]

            def get_st16(oh, j, pi):
                key = (oh, j) if pi == 0 else (oh, j, pi)
                if key not in st16:
                    wt = wpool.tile([P, 4, ON], bf16, tag="wt16",
                                    name=f"wt16_{pi}_{j}")
                    nc.gpsimd.dma_start(out=wt, in_=w16_d[oh, j])
                    st16[key] = wt
                return st16[key]

            def get_st8(oh, j, pi):
                key = (oh, j) if pi == 0 else (oh, j, pi)
                if key not in st8:
                    wt = w8pool.tile([P, 4, 2, ON], f8, tag="wt8",
                                     name=f"wt8_{pi}_{j}")
                    nc.gpsimd.dma_start(out=wt, in_=w8_d[oh, j])
                    st8[key] = wt
                return st8[key]

            for pi, (oh, blo, bhi) in enumerate(passes):
                n16 = 0
                n8 = 0
                for ci, (kind, idx) in enumerate(order):
                    st = (ci == 0)
                    sp = (ci == nchunk - 1)
                    if kind == "b":
                        wt = get_st16(oh, n16 // 4, pi)
                        c = n16 % 4
                        n16 += 1
                        ic, di = divmod(idx, NB16)
                        bt = b16[ic]
                        for b in range(blo, bhi):
                            nc.tensor.matmul(psums[b],
                                             bt[:, di, b * P:(b + 1) * P],
                                             wt[:, c, :],
                                             start=st, stop=sp)
                    else:
                        wt8 = get_st8(oh, n8 // 4, pi)
                        c = n8 % 4
                        n8 += 1
                        pt = pairs[idx]
                        for b in range(blo, bhi):
                            nc.tensor.matmul(psums[b],
                                             pt[:, :, b * P:(b + 1) * P],
                                             wt8[:, c, :, :],
                                             start=st, stop=sp, perf_mode=DR)

                for b in range(blo, bhi):
                    ot = opool.tile([P, ON], f32, tag="ot", name=f"ot_{pi}_{b}")
                    bias_sl = bias_t[:, oh * ON:(oh + 1) * ON]
                    if pi == 0 and b < 4:
                        # banks 0-3 gate pass 1a: drain via ACT copy (with the
                        # 2^-14 weight descale) so the start=True matmuls
                        # aren't stuck behind the serial DVE chain; add bias
                        # in place later (overlaps the next pass).
                        nc.scalar.activation(ot, psums[b], AF.Copy,
                                             scale=WSINV)
                        nc.vector.tensor_add(ot, ot, bias_sl)
                        nc.sync.dma_start(
                            out=y_d[b * P:(b + 1) * P,
                                    oh * ON:(oh + 1) * ON],
                            in_=ot)
                    elif pi == len(passes) - 1:
                        # final pass: split banks across ACT and DVE so the
                        # tail drains on two engines in parallel, at half-
                        # granularity so stores overlap the other half's add
                        for hh in range(2):
                            hsl = slice(hh * (ON // 2), (hh + 1) * (ON // 2))
                            if b % 2 == 0:
                                nc.scalar.activation(ot[:, hsl],
                                                     psums[b][:, hsl],
                                                     AF.Copy, scale=WSINV)
                                nc.vector.tensor_add(ot[:, hsl], ot[:, hsl],
                                                     bias_sl[:, hsl])
                            else:
                                nc.vector.scalar_tensor_tensor(
                                    ot[:, hsl], psums[b][:, hsl], WSINV,
                                    bias_sl[:, hsl], ALU.mult, ALU.add)
                            nc.sync.dma_start(
                                out=y_d[b * P:(b + 1) * P,
                                        oh * ON + hh * (ON // 2):
                                        oh * ON + (hh + 1) * (ON // 2)],
                                in_=ot[:, hsl])
                    else:
                        nc.vector.scalar_tensor_tensor(
                            ot, psums[b], WSINV, bias_sl, ALU.mult, ALU.add)
                        nc.sync.dma_start(
                            out=y_d[b * P:(b + 1) * P,
                                    oh * ON:(oh + 1) * ON],
                            in_=ot)
    nc.compile()  # bacc legalization: splits multi-sem waits (TRN2 allows 1)
    return nc


def _get_nc():
    global _nc
    if _nc is None:
        _nc = _build_nc()
    return _nc


def _prep_inputs(x, cheby_coeffs):
    x = np.asarray(x, dtype=np.float32)
    C = np.asarray(cheby_coeffs, dtype=np.float32)
    bf16 = ml_dtypes.bfloat16
    e4 = ml_dtypes.float8_e4m3

    Wd = C[:, :, 1:] * np.float32(WSCALE)              # [I, O, 8], d index 0..7
    Wd = Wd.reshape(IC, P, OH, ON, 8)                  # [ic, p, oh, on, d]

    # bf16 chunks: degrees 4..8, k16 = ic*5 + (d-4); packed [oh,st,p,4,on]
    W16 = np.transpose(Wd[:, :, :, :, 3:8], (2, 0, 4, 1, 3))  # [oh,ic,dd,p,on]
    W16 = W16.reshape(OH, NK16, P, ON).reshape(OH, NST16, 4, P, ON)
    W16 = np.ascontiguousarray(np.transpose(W16, (0, 1, 3, 2, 4))).astype(bf16)

    # fp8 pairs: A[ic] slots (d=1, d=2); B[j] slots (d=3 @ ic=2j, 2j+1);
    # packed [oh, st, p, 4, 2, on]
    W8 = np.empty((OH, NP8, P, 2, ON), np.float32)
    for ic in range(IC):
        W8[:, ic, :, 0, :] = np.transpose(Wd[ic, :, :, :, 0], (1, 0, 2))
        W8[:, ic, :, 1, :] = np.transpose(Wd[ic, :, :, :, 1], (1, 0, 2))
    for j in range(IC // 2):
        W8[:, IC + j, :, 0, :] = np.transpose(Wd[2 * j, :, :, :, 2], (1, 0, 2))
        W8[:, IC + j, :, 1, :] = np.transpose(Wd[2 * j + 1, :, :, :, 2],
                                              (1, 0, 2))
    # reorder pairs into the schedule sequence used by _chunk_order()
    p8_seq = [idx for kind, idx in _chunk_order() if kind == "p"]
    W8 = W8[:, p8_seq]
    W8 = W8.reshape(OH, NST8, 4, P, 2, ON)
    W8 = np.ascontiguousarray(np.transpose(W8, (0, 1, 3, 2, 4, 5))).astype(e4)

    bias = C[:, :, 0].sum(axis=0, dtype=np.float64).astype(np.float32)
    bias_rep = np.ascontiguousarray(np.broadcast_to(bias, (P, O_DIM)))

    in_maps = []
    for c in range(N_CORES):
        xt = np.ascontiguousarray(x[c * B_LOCAL:(c + 1) * B_LOCAL, :].T)
        in_maps.append({"xt": xt, "w16": W16, "w8": W8, "bias": bias_rep})
    return in_maps


def kernel(x, cheby_coeffs):
    global last_results
    nc = _get_nc()
    in_maps = _prep_inputs(x, cheby_coeffs)
    last_results = run_bass_kernel_spmd(nc, in_maps,
                                        core_ids=list(range(N_CORES)))
    y = np.concatenate([r["y"] for r in last_results.results], axis=0)
    return y


# revision 14
# speedup vs baseline: 1.0092x; 1.0092x over previous
"""ChebyKAN layer kernel for 8x Trainium2 NeuronCores.

Computes y[b,o] = sum_{i,d} T_d(tanh(x[b,i])) * C[i,o,d], d = 0..8, via:
  - batch sharded 8 ways (1024 rows/core)
  - fp32 Chebyshev recurrence for T_1..T_8 on ACT/DVE
  - d=0 term (T_0 == 1) folded into a host-precomputed bias[o]
  - HYBRID contraction, K = (i,d) of size 8192:
      degrees {1,2,3} (24 K-chunks) as fp8(e4m3) DoubleRow matmuls --
      2 K-chunks per instruction at 2x throughput;
      degrees {4..8} (40 K-chunks) as bf16 matmuls.
    All weights are pre-scaled by 2^14 (exact power of two) so e4m3
    holds them with full mantissa precision and BOTH dtypes accumulate
    at the same scale into the same PSUM bank; the 2^-14 is folded into
    the bias-add at PSUM drain. Max rel err vs fp32 reference ~1.7e-2
    (tolerance 2e-2).
  - weights stream as 4-chunk "supertiles" (one DMA descriptor per 4
    K-chunks) so the DMA issue rate never starves the PE
  - x is transposed on host so the basis is produced directly in
    [K, batch] (lhsT) layout; no on-device transpose needed.

Self-contained: hardcodes all shapes for inputs
  x: [8192, 1024] f32, cheby_coeffs: [1024, 1024, 9] f32.
"""

import numpy as np
import ml_dtypes

import concourse.bass as bass
import concourse.mybir as mybir
import concourse.tile as tile
from concourse import bacc
from concourse.bass_utils import run_bass_kernel_spmd

P = 128
B_TOTAL = 8192
I_DIM = 1024
O_DIM = 1024
N_CORES = 8
B_LOCAL = B_TOTAL // N_CORES     # 1024
IC = I_DIM // P                  # 8 input chunks
OH = 2                           # output halves (PSUM bank = 512 fp32)
ON = O_DIM // OH                 # 512

NB16 = 5                         # bf16 degrees 4..8
NK16 = IC * NB16                 # 40 bf16 K-chunks
NST16 = NK16 // 4                # 10 bf16 supertiles (4 chunks each)
NP8 = IC + IC // 2               # 12 fp8 pairs: A[ic]=(d1,d2), B[j]=(d3,d3)
NST8 = NP8 // 4                  # 3 fp8 supertiles (4 pairs each)
WSCALE = 2.0 ** 14               # weight pre-scale (exact, shared by dtypes)
WSINV = 2.0 ** -14

_nc = None
last_results = None  # BassKernelResults of the most recent run (for profiling)


def _ensure_ntff_hook():
    """bass_utils' trace path imports antenv.axon_hooks unconditionally, but
    this agent image's antenv package lacks that module. Synthesize it (with
    the real libaxon NTFF hook when available) so a BASS_TRACE=1 run traces
    instead of crashing."""
    import sys
    import types

    try:
        import antenv.axon_hooks  # noqa: F401
        return
    except ImportError:
        pass
    try:
        import antenv
    except ImportError:
        return
    hook = None
    try:
        from trn_agent_boot.trn_boot import _ntff_profile_via_ctypes
        hook = _ntff_profile_via_ctypes("/opt/axon/libaxon_pjrt.so")
    except Exception:
        hook = None
    mod = types.ModuleType("antenv.axon_hooks")
    state = {"hook": hook}
    mod.set_axon_ntff_profile_hook = lambda h: state.__setitem__("hook", h)
    mod.get_axon_ntff_profile_hook = lambda: state["hook"]
    sys.modules["antenv.axon_hooks"] = mod
    antenv.axon_hooks = mod


_ensure_ntff_hook()


def _chunk_order():
    """Per-o-half K-chunk schedule: ("b", k16) bf16 chunks and ("p", pid)
    fp8 pairs, ordered so every chunk's basis is ready shortly after its
    ic's recurrence runs. Pair A[ic]=(T1,T2) leads each ic (earliest
    ready); pair B[j]=(T3@2j, T3@2j+1) follows ic=2j+1."""
    order = []
    for ic in range(IC):
        order.append(("p", ic))
        for di in range(NB16):
            order.append(("b", ic * NB16 + di))
        if ic % 2 == 1:
            order.append(("p", IC + ic // 2))
    return order


def _build_nc():
    nc = bacc.Bacc()
    f32 = mybir.dt.float32
    bf16 = mybir.dt.bfloat16
    f8 = mybir.dt.float8e4
    AF = mybir.ActivationFunctionType
    ALU = mybir.AluOpType
    DR = mybir.MatmulPerfMode.DoubleRow

    xt_d = nc.dram_tensor("xt", [I_DIM, B_LOCAL], f32, kind="ExternalInput")
    w16_d = nc.dram_tensor("w16", [OH, NST16, P, 4, ON], bf16,
                           kind="ExternalInput")
    w8_d = nc.dram_tensor("w8", [OH, NST8, P, 4, 2, ON], f8,
                          kind="ExternalInput")
    bias_d = nc.dram_tensor("bias", [P, O_DIM], f32, kind="ExternalInput")
    y_d = nc.dram_tensor("y", [B_LOCAL, O_DIM], f32, kind="ExternalOutput")

    order = _chunk_order()
    nchunk = len(order)

    with tile.TileContext(nc) as tc:
        with (
            tc.tile_pool(name="const", bufs=1) as cpool,
            tc.tile_pool(name="xin", bufs=2) as xpool,
            tc.tile_pool(name="fwork", bufs=2) as fpool,
            tc.tile_pool(name="basis", bufs=1) as bpool,
            tc.tile_pool(name="w16s", bufs=4) as wpool,
            tc.tile_pool(name="w8s", bufs=2) as w8pool,
            tc.tile_pool(name="outbuf", bufs=4) as opool,
            tc.tile_pool(name="acc", bufs=1, space="PSUM") as ppool,
        ):
            # ---- first xt half + per-chunk prefetch of the first weight
            # supertiles. The sync queue's wire rate (~140 GB/s) makes DMA
            # ORDER matter: the basis chain is gated on xt0a, and the first
            # matmuls only need chunk 0 of each supertile, so those 128KB
            # pieces go ahead of everything bulky.
            st16 = {}
            st8 = {}
            xt0 = xpool.tile([P, B_LOCAL], f32, tag="xt", name="xt_0")
            nc.sync.dma_start(out=xt0, in_=xt_d[0:P, :])
            st8[(0, 0)] = w8pool.tile([P, 4, 2, ON], f8, tag="wt8",
                                      name="wt8_p0")
            nc.sync.dma_start(out=st8[(0, 0)][:, 0, :, :], in_=w8_d[0, 0, :, 0])
            st16[(0, 0)] = wpool.tile([P, 4, ON], bf16, tag="wt16",
                                      name="wt16_p0")
            nc.sync.dma_start(out=st16[(0, 0)][:, 0, :], in_=w16_d[0, 0, :, 0])
            for c in range(1, 4):
                nc.sync.dma_start(out=st16[(0, 0)][:, c, :],
                                  in_=w16_d[0, 0, :, c])
            for c in range(1, 4):
                nc.sync.dma_start(out=st8[(0, 0)][:, c, :, :],
                                  in_=w8_d[0, 0, :, c])

            # ---- PE warm-up ----
            # HAM un-throttles the PE clock (1.2 -> 2.4 GHz) only after
            # ~3.4us of sustained matmul activity; bridge until the first
            # basis chunk is ready (~12us) with dummy N=512 matmuls. memset
            # on gpsimd: its queue clears the trace barrier earliest.
            warm = cpool.tile([P, ON], bf16, name="warm")
            nc.gpsimd.memset(warm, 1.0)
            warm_ps = ppool.tile([P, ON], f32, tag="ps0", name="warm_ps")
            for wi in range(8):
                nc.tensor.matmul(warm_ps, warm[:, 0:P], warm,
                                 start=(wi == 0), stop=(wi == 7))

            # ---- basis production per 128-row chunk of i ----
            # pairs A[ic]: [128, 2, B] e4m3, slots (T1, T2) of ic
            # pairs B[j]:  [128, 2, B] e4m3, slots (T3@ic=2j, T3@ic=2j+1)
            # bf16[ic]:    [128, 5, B] bf16, slots (T4..T8) of ic
            # one tag per family with bufs = #tiles: same memory, far fewer
            # semaphores (the end-of-kernel semaphore-reset epilogue is
            # serialized per distinct semaphore).
            pairs = {}
            for ic in range(IC):
                pairs[ic] = bpool.tile([P, 2, B_LOCAL], f8, tag="pA",
                                       bufs=IC, name=f"pA{ic}")
            for j in range(IC // 2):
                pairs[IC + j] = bpool.tile([P, 2, B_LOCAL], f8, tag="pB",
                                           bufs=IC // 2, name=f"pB{j}")
            b16 = {}
            for ic in range(IC):
                b16[ic] = bpool.tile([P, NB16, B_LOCAL], bf16, tag="b16",
                                     bufs=IC, name=f"b{ic}")

            for ic in range(IC):
                # ic == 0 runs every op on two half-tiles so the first
                # K-chunks are ready earlier (overlaps PE ramp-up).
                slices = ([slice(0, B_LOCAL // 2), slice(B_LOCAL // 2, B_LOCAL)]
                          if ic == 0 else [slice(0, B_LOCAL)])

                if ic == 0:
                    xt_t = xt0  # prefetched above
                else:
                    # full-tile DMA (4KB/partition contiguous beats two 2KB
                    # halves on wire efficiency), all on the sync queue: the
                    # gpsimd queue must stay clear for the weight supertile
                    # stream, which pass 0 consumes at a much higher rate
                    # than the basis chain consumes xt.
                    xt_t = xpool.tile([P, B_LOCAL], f32, tag="xt",
                                      name=f"xt_{ic}")
                    nc.sync.dma_start(out=xt_t,
                                      in_=xt_d[ic * P:(ic + 1) * P, :])

                pA = pairs[ic]
                pB = pairs[IC + ic // 2]
                bb = b16[ic]

                t = fpool.tile([P, B_LOCAL], f32, tag="T1", name=f"t_{ic}")
                s2 = fpool.tile([P, B_LOCAL], f32, tag="sq", name=f"s2_{ic}")
                T2 = fpool.tile([P, B_LOCAL], f32, tag="Tn", name=f"T2_{ic}",
                                bufs=3)
                V3 = fpool.tile([P, B_LOCAL], f32, tag="u", name=f"V3_{ic}")
                T3 = fpool.tile([P, B_LOCAL], f32, tag="Tn", name=f"T3_{ic}",
                                bufs=3)
                s4 = fpool.tile([P, B_LOCAL], f32, tag="sq", name=f"s4_{ic}")
                T4 = fpool.tile([P, B_LOCAL], f32, tag="Tn", name=f"T4_{ic}",
                                bufs=3)
                u5 = fpool.tile([P, B_LOCAL], f32, tag="u", name=f"u5_{ic}")
                s6 = fpool.tile([P, B_LOCAL], f32, tag="sq", name=f"s6_{ic}")
                u7 = fpool.tile([P, B_LOCAL], f32, tag="u", name=f"u7_{ic}")
                s8 = fpool.tile([P, B_LOCAL], f32, tag="sq", name=f"s8_{ic}")

                for sl in slices:
                    # T1 = tanh(x) (no clip: recurrence stable for |t|<=1;
                    # deviation from the reference's 0.999 clip is ~1e-6)
                    nc.scalar.activation(t[:, sl], xt_t[:, sl], AF.Tanh)
                    nc.vector.tensor_copy(pA[:, 0, sl], t[:, sl])

                    # T2 = 2 t^2 - 1
                    nc.scalar.square(s2[:, sl], t[:, sl])
                    nc.vector.tensor_scalar(T2[:, sl], s2[:, sl], 2.0, -1.0,
                                            ALU.mult, ALU.add)
                    nc.scalar.copy(pA[:, 1, sl], T2[:, sl])

                    # T3 = t * (2 T2 - 1)
                    nc.vector.tensor_scalar(V3[:, sl], T2[:, sl], 2.0, -1.0,
                                            ALU.mult, ALU.add)
                    nc.vector.tensor_mul(T3[:, sl], t[:, sl], V3[:, sl])
                    nc.scalar.copy(pB[:, ic % 2, sl], T3[:, sl])

                    # T4 = 2 T2^2 - 1
                    nc.scalar.square(s4[:, sl], T2[:, sl])
                    nc.vector.tensor_scalar(T4[:, sl], s4[:, sl], 2.0, -1.0,
                                            ALU.mult, ALU.add)
                    nc.scalar.copy(bb[:, 0, sl], T4[:, sl])

                    # T5 = 2 T2 T3 - T1, T7 = 2 T3 T4 - T1 (fp32 products so
                    # the bf16 basis keeps full bf16 accuracy)
                    nc.vector.tensor_mul(u5[:, sl], T2[:, sl], T3[:, sl])
                    nc.vector.scalar_tensor_tensor(bb[:, 1, sl], u5[:, sl],
                                                   2.0, t[:, sl],
                                                   ALU.mult, ALU.subtract)

                    # T6 = 2 T3^2 - 1, T8 = 2 T4^2 - 1
                    nc.scalar.square(s6[:, sl], T3[:, sl])
                    nc.vector.tensor_scalar(bb[:, 2, sl], s6[:, sl], 2.0, -1.0,
                                            ALU.mult, ALU.add)

                    nc.vector.tensor_mul(u7[:, sl], T3[:, sl], T4[:, sl])
                    nc.vector.scalar_tensor_tensor(bb[:, 3, sl], u7[:, sl],
                                                   2.0, t[:, sl],
                                                   ALU.mult, ALU.subtract)

                    nc.scalar.square(s8[:, sl], T4[:, sl])
                    nc.vector.tensor_scalar(bb[:, 4, sl], s8[:, sl], 2.0, -1.0,
                                            ALU.mult, ALU.add)

            # bias is only consumed at the end of each pass; load it late so
            # it doesn't delay the xt/wt streams.
            bias_t = cpool.tile([P, O_DIM], f32, name="bias_t")
            nc.sync.dma_start(out=bias_t, in_=bias_d[:, :])

            # ---- contraction passes ----
            psums = [ppool.tile([P, ON], f32, tag=f"ps{b}", name=f"ps{b}")
                     for b in range(B_LOCAL // P)]
            # pass 0: o-half 0, all 8 batch banks (overlaps basis production)
            # pass 1a/1b: o-half 1 split in two bank halves, so the first
            # half's drains + stores overlap the second half's matmuls.
            passes = [(0, 0, 8), (1, 0, 4), (1, 4, 8)]

            def get_st16(oh, j, pi):
                key = (oh, j) if pi == 0 else (oh, j, pi)
                if key not in st16:
                    wt = wpool.tile([P, 4, ON], bf16, tag="wt16",
                                    name=f"wt16_{pi}_{j}")
                    nc.gpsimd.dma_start(out=wt, in_=w16_d[oh, j])
                    st16[key] = wt
                return st16[key]

            def get_st8(oh, j, pi):
                key = (oh, j) if pi == 0 else (oh, j, pi)
                if key not in st8:
                    wt = w8pool.tile([P, 4, 2, ON], f8, tag="wt8",
                                     name=f"wt8_{pi}_{j}")
                    nc.gpsimd.dma_start(out=wt, in_=w8_d[oh, j])
                    st8[key] = wt
                return st8[key]

            for pi, (oh, blo, bhi) in enumerate(passes):
                n16 = 0
                n8 = 0
                for ci, (kind, idx) in enumerate(order):
                    st = (ci == 0)
                    sp = (ci == nchunk - 1)
                    if kind == "b":
                        wt = get_st16(oh, n16 // 4, pi)
                        c = n16 % 4
                        n16 += 1
                        ic, di = divmod(idx, NB16)
                        bt = b16[ic]
                        for b in range(blo, bhi):
                            nc.tensor.matmul(psums[b],
                                             bt[:, di, b * P:(b + 1) * P],
                                             wt[:, c, :],
                                             start=st, stop=sp)
                    else:
                        wt8 = get_st8(oh, n8 // 4, pi)
                        c = n8 % 4
                        n8 += 1
                        pt = pairs[idx]
                        for b in range(blo, bhi):
                            nc.tensor.matmul(psums[b],
                                             pt[:, :, b * P:(b + 1) * P],
                                             wt8[:, c, :, :],
                                             start=st, stop=sp, perf_mode=DR)

                for b in range(blo, bhi):
                    ot = opool.tile([P, ON], f32, tag="ot", name=f"ot_{pi}_{b}")
                    bias_sl = bias_t[:, oh * ON:(oh + 1) * ON]
                    if pi == 0 and b < 4:
                        # banks 0-3 gate pass 1a: drain via ACT copy (with the
                        # 2^-14 weight descale) so the start=True matmuls
                        # aren't stuck behind the serial DVE chain; add bias
                        # in place later (overlaps the next pass).
                        nc.scalar.activation(ot, psums[b], AF.Copy,
                                             scale=WSINV)
                        nc.vector.tensor_add(ot, ot, bias_sl)
                        nc.sync.dma_start(
                            out=y_d[b * P:(b + 1) * P,
                                    oh * ON:(oh + 1) * ON],
                            in_=ot)
                    elif pi == len(passes) - 1:
                        # final pass: split banks across ACT and DVE so the
                        # tail drains on two engines in parallel, at half-
                        # granularity so stores overlap the other half's add
                        for hh in range(2):
                            hsl = slice(hh * (ON // 2), (hh + 1) * (ON // 2))
                            if b % 2 == 0:
                                nc.scalar.activation(ot[:, hsl],
                                                     psums[b][:, hsl],
                                                     AF.Copy, scale=WSINV)
                                nc.vector.tensor_add(ot[:, hsl], ot[:, hsl],
                                                     bias_sl[:, hsl])
                            else:
                                nc.vector.scalar_tensor_tensor(
                                    ot[:, hsl], psums[b][:, hsl], WSINV,
                                    bias_sl[:, hsl], ALU.mult, ALU.add)
                            # alternate store queues: 8 back-to-back issues
                            # (~700ns each) would otherwise serialize the tail
                            q = nc.sync if b % 2 == 0 else nc.gpsimd
                            q.dma_start(
                                out=y_d[b * P:(b + 1) * P,
                                        oh * ON + hh * (ON // 2):
                                        oh * ON + (hh + 1) * (ON // 2)],
                                in_=ot[:, hsl])
                    else:
                        nc.vector.scalar_tensor_tensor(
                            ot, psums[b], WSINV, bias_sl, ALU.mult, ALU.add)
                        nc.sync.dma_start(
                            out=y_d[b * P:(b + 1) * P,
                                    oh * ON:(oh + 1) * ON],
                            in_=ot)
    nc.compile()  # bacc legalization: splits multi-sem waits (TRN2 allows 1)
    return nc


def _get_nc():
    global _nc
    if _nc is None:
        _nc = _build_nc()
    return _nc


def _prep_inputs(x, cheby_coeffs):
    x = np.asarray(x, dtype=np.float32)
    C = np.asarray(cheby_coeffs, dtype=np.float32)
    bf16 = ml_dtypes.bfloat16
    e4 = ml_dtypes.float8_e4m3

    Wd = C[:, :, 1:] * np.float32(WSCALE)              # [I, O, 8], d index 0..7
    Wd = Wd.reshape(IC, P, OH, ON, 8)                  # [ic, p, oh, on, d]

    # bf16 chunks: degrees 4..8, k16 = ic*5 + (d-4); packed [oh,st,p,4,on]
    W16 = np.transpose(Wd[:, :, :, :, 3:8], (2, 0, 4, 1, 3))  # [oh,ic,dd,p,on]
    W16 = W16.reshape(OH, NK16, P, ON).reshape(OH, NST16, 4, P, ON)
    W16 = np.ascontiguousarray(np.transpose(W16, (0, 1, 3, 2, 4))).astype(bf16)

    # fp8 pairs: A[ic] slots (d=1, d=2); B[j] slots (d=3 @ ic=2j, 2j+1);
    # packed [oh, st, p, 4, 2, on]
    W8 = np.empty((OH, NP8, P, 2, ON), np.float32)
    for ic in range(IC):
        W8[:, ic, :, 0, :] = np.transpose(Wd[ic, :, :, :, 0], (1, 0, 2))
        W8[:, ic, :, 1, :] = np.transpose(Wd[ic, :, :, :, 1], (1, 0, 2))
    for j in range(IC // 2):
        W8[:, IC + j, :, 0, :] = np.transpose(Wd[2 * j, :, :, :, 2], (1, 0, 2))
        W8[:, IC + j, :, 1, :] = np.transpose(Wd[2 * j + 1, :, :, :, 2],
                                              (1, 0, 2))
    # reorder pairs into the schedule sequence used by _chunk_order()
    p8_seq = [idx for kind, idx in _chunk_order() if kind == "p"]
    W8 = W8[:, p8_seq]
    W8 = W8.reshape(OH, NST8, 4, P, 2, ON)
    W8 = np.ascontiguousarray(np.transpose(W8, (0, 1, 3, 2, 4, 5))).astype(e4)

    bias = C[:, :, 0].sum(axis=0, dtype=np.float64).astype(np.float32)
    bias_rep = np.ascontiguousarray(np.broadcast_to(bias, (P, O_DIM)))

    in_maps = []
    for c in range(N_CORES):
        xt = np.ascontiguousarray(x[c * B_LOCAL:(c + 1) * B_LOCAL, :].T)
        in_maps.append({"xt": xt, "w16": W16, "w8": W8, "bias": bias_rep})
    return in_maps


def kernel(x, cheby_coeffs):
    global last_results
    nc = _get_nc()
    in_maps = _prep_inputs(x, cheby_coeffs)
    last_results = run_bass_kernel_spmd(nc, in_maps,
                                        core_ids=list(range(N_CORES)))
    y = np.concatenate([r["y"] for r in last_results.results], axis=0)
    return y


# revision 15
# speedup vs baseline: 1.0121x; 1.0029x over previous
"""ChebyKAN layer kernel for 8x Trainium2 NeuronCores.

Computes y[b,o] = sum_{i,d} T_d(tanh(x[b,i])) * C[i,o,d], d = 0..8, via:
  - batch sharded 8 ways (1024 rows/core)
  - fp32 Chebyshev recurrence for T_1..T_8 on ACT/DVE
  - d=0 term (T_0 == 1) folded into a host-precomputed bias[o]
  - HYBRID contraction, K = (i,d) of size 8192:
      degrees {1,2,3} (24 K-chunks) as fp8(e4m3) DoubleRow matmuls --
      2 K-chunks per instruction at 2x throughput;
      degrees {4..8} (40 K-chunks) as bf16 matmuls.
    All weights are pre-scaled by 2^14 (exact power of two) so e4m3
    holds them with full mantissa precision and BOTH dtypes accumulate
    at the same scale into the same PSUM bank; the 2^-14 is folded into
    the bias-add at PSUM drain. Max rel err vs fp32 reference ~1.7e-2
    (tolerance 2e-2).
  - weights stream as 4-chunk "supertiles" (one DMA descriptor per 4
    K-chunks) so the DMA issue rate never starves the PE
  - x is transposed on host so the basis is produced directly in
    [K, batch] (lhsT) layout; no on-device transpose needed.

Self-contained: hardcodes all shapes for inputs
  x: [8192, 1024] f32, cheby_coeffs: [1024, 1024, 9] f32.
"""

import numpy as np
import ml_dtypes

import concourse.bass as bass
import concourse.mybir as mybir
import concourse.tile as tile
from concourse import bacc
from concourse.bass_utils import run_bass_kernel_spmd

P = 128
B_TOTAL = 8192
I_DIM = 1024
O_DIM = 1024
N_CORES = 8
B_LOCAL = B_TOTAL // N_CORES     # 1024
IC = I_DIM // P                  # 8 input chunks
OH = 2                           # output halves (PSUM bank = 512 fp32)
ON = O_DIM // OH                 # 512

NB16 = 5                         # bf16 degrees 4..8
NK16 = IC * NB16                 # 40 bf16 K-chunks
NST16 = NK16 // 4                # 10 bf16 supertiles (4 chunks each)
NP8 = IC + IC // 2               # 12 fp8 pairs: A[ic]=(d1,d2), B[j]=(d3,d3)
NST8 = NP8 // 4                  # 3 fp8 supertiles (4 pairs each)
WSCALE = 2.0 ** 14               # weight pre-scale (exact, shared by dtypes)
WSINV = 2.0 ** -14

_nc = None
last_results = None  # BassKernelResults of the most recent run (for profiling)


def _ensure_ntff_hook():
    """bass_utils' trace path imports antenv.axon_hooks unconditionally, but
    this agent image's antenv package lacks that module. Synthesize it (with
    the real libaxon NTFF hook when available) so a BASS_TRACE=1 run traces
    instead of crashing."""
    import sys
    import types

    try:
        import antenv.axon_hooks  # noqa: F401
        return
    except ImportError:
        pass
    try:
        import antenv
    except ImportError:
        return
    hook = None
    try:
        from trn_agent_boot.trn_boot import _ntff_profile_via_ctypes
        hook = _ntff_profile_via_ctypes("/opt/axon/libaxon_pjrt.so")
    except Exception:
        hook = None
    mod = types.ModuleType("antenv.axon_hooks")
    state = {"hook": hook}
    mod.set_axon_ntff_profile_hook = lambda h: state.__setitem__("hook", h)
    mod.get_axon_ntff_profile_hook = lambda: state["hook"]
    sys.modules["antenv.axon_hooks"] = mod
    antenv.axon_hooks = mod


_ensure_ntff_hook()


def _chunk_order():
    """Per-o-half K-chunk schedule: ("b", k16) bf16 chunks and ("p", pid)
    fp8 pairs, ordered so every chunk's basis is ready shortly after its
    ic's recurrence runs. Pair A[ic]=(T1,T2) leads each ic (earliest
    ready); pair B[j]=(T3@2j, T3@2j+1) follows ic=2j+1."""
    order = []
    for ic in range(IC):
        order.append(("p", ic))
        for di in range(NB16):
            order.append(("b", ic * NB16 + di))
        if ic % 2 == 1:
            order.append(("p", IC + ic // 2))
    return order


def _build_nc():
    nc = bacc.Bacc()
    f32 = mybir.dt.float32
    bf16 = mybir.dt.bfloat16
    f8 = mybir.dt.float8e4
    AF = mybir.ActivationFunctionType
    ALU = mybir.AluOpType
    DR = mybir.MatmulPerfMode.DoubleRow

    xt_d = nc.dram_tensor("xt", [I_DIM, B_LOCAL], f32, kind="ExternalInput")
    w16_d = nc.dram_tensor("w16", [OH, NST16, P, 4, ON], bf16,
                           kind="ExternalInput")
    w8_d = nc.dram_tensor("w8", [OH, NST8, P, 4, 2, ON], f8,
                          kind="ExternalInput")
    bias_d = nc.dram_tensor("bias", [P, O_DIM], f32, kind="ExternalInput")
    y_d = nc.dram_tensor("y", [B_LOCAL, O_DIM], f32, kind="ExternalOutput")

    order = _chunk_order()
    nchunk = len(order)

    with tile.TileContext(nc) as tc:
        with (
            tc.tile_pool(name="const", bufs=1) as cpool,
            tc.tile_pool(name="xin", bufs=2) as xpool,
            tc.tile_pool(name="fwork", bufs=2) as fpool,
            tc.tile_pool(name="basis", bufs=1) as bpool,
            tc.tile_pool(name="w16s", bufs=4) as wpool,
            tc.tile_pool(name="w8s", bufs=2) as w8pool,
            tc.tile_pool(name="outbuf", bufs=4) as opool,
            tc.tile_pool(name="acc", bufs=1, space="PSUM") as ppool,
        ):
            # ---- first xt half + per-chunk prefetch of the first weight
            # supertiles. The sync queue's wire rate (~140 GB/s) makes DMA
            # ORDER matter: the basis chain is gated on xt0a, and the first
            # matmuls only need chunk 0 of each supertile, so those 128KB
            # pieces go ahead of everything bulky.
            st16 = {}
            st8 = {}
            xt0 = xpool.tile([P, B_LOCAL], f32, tag="xt", name="xt_0")
            for sl in (slice(0, B_LOCAL // 2), slice(B_LOCAL // 2, B_LOCAL)):
                nc.sync.dma_start(out=xt0[:, sl], in_=xt_d[0:P, sl])
            st8[(0, 0)] = w8pool.tile([P, 4, 2, ON], f8, tag="wt8",
                                      name="wt8_p0")
            nc.sync.dma_start(out=st8[(0, 0)][:, 0, :, :], in_=w8_d[0, 0, :, 0])
            st16[(0, 0)] = wpool.tile([P, 4, ON], bf16, tag="wt16",
                                      name="wt16_p0")
            nc.sync.dma_start(out=st16[(0, 0)][:, 0, :], in_=w16_d[0, 0, :, 0])
            for c in range(1, 4):
                nc.sync.dma_start(out=st16[(0, 0)][:, c, :],
                                  in_=w16_d[0, 0, :, c])
            for c in range(1, 4):
                nc.sync.dma_start(out=st8[(0, 0)][:, c, :, :],
                                  in_=w8_d[0, 0, :, c])

            # ---- PE warm-up ----
            # HAM un-throttles the PE clock (1.2 -> 2.4 GHz) only after
            # ~3.4us of sustained matmul activity; bridge until the first
            # basis chunk is ready (~12us) with dummy N=512 matmuls. memset
            # on gpsimd: its queue clears the trace barrier earliest.
            warm = cpool.tile([P, ON], bf16, name="warm")
            nc.gpsimd.memset(warm, 1.0)
            warm_ps = ppool.tile([P, ON], f32, tag="ps0", name="warm_ps")
            for wi in range(8):
                nc.tensor.matmul(warm_ps, warm[:, 0:P], warm,
                                 start=(wi == 0), stop=(wi == 7))

            # ---- basis production per 128-row chunk of i ----
            # pairs A[ic]: [128, 2, B] e4m3, slots (T1, T2) of ic
            # pairs B[j]:  [128, 2, B] e4m3, slots (T3@ic=2j, T3@ic=2j+1)
            # bf16[ic]:    [128, 5, B] bf16, slots (T4..T8) of ic
            # one tag per family with bufs = #tiles: same memory, far fewer
            # semaphores (the end-of-kernel semaphore-reset epilogue is
            # serialized per distinct semaphore).
            pairs = {}
            for ic in range(IC):
                pairs[ic] = bpool.tile([P, 2, B_LOCAL], f8, tag="pA",
                                       bufs=IC, name=f"pA{ic}")
            for j in range(IC // 2):
                pairs[IC + j] = bpool.tile([P, 2, B_LOCAL], f8, tag="pB",
                                           bufs=IC // 2, name=f"pB{j}")
            b16 = {}
            for ic in range(IC):
                b16[ic] = bpool.tile([P, NB16, B_LOCAL], bf16, tag="b16",
                                     bufs=IC, name=f"b{ic}")

            for ic in range(IC):
                # ic == 0 runs every op on two half-tiles so the first
                # K-chunks are ready earlier (overlaps PE ramp-up).
                slices = ([slice(0, B_LOCAL // 2), slice(B_LOCAL // 2, B_LOCAL)]
                          if ic == 0 else [slice(0, B_LOCAL)])

                if ic == 0:
                    xt_t = xt0  # prefetched above
                else:
                    # full-tile DMA (4KB/partition contiguous beats two 2KB
                    # halves on wire efficiency), all on the sync queue: the
                    # gpsimd queue must stay clear for the weight supertile
                    # stream, which pass 0 consumes at a much higher rate
                    # than the basis chain consumes xt.
                    xt_t = xpool.tile([P, B_LOCAL], f32, tag="xt",
                                      name=f"xt_{ic}")
                    nc.sync.dma_start(out=xt_t,
                                      in_=xt_d[ic * P:(ic + 1) * P, :])

                pA = pairs[ic]
                pB = pairs[IC + ic // 2]
                bb = b16[ic]

                t = fpool.tile([P, B_LOCAL], f32, tag="T1", name=f"t_{ic}")
                s2 = fpool.tile([P, B_LOCAL], f32, tag="sq", name=f"s2_{ic}")
                T2 = fpool.tile([P, B_LOCAL], f32, tag="Tn", name=f"T2_{ic}",
                                bufs=3)
                V3 = fpool.tile([P, B_LOCAL], f32, tag="u", name=f"V3_{ic}")
                T3 = fpool.tile([P, B_LOCAL], f32, tag="Tn", name=f"T3_{ic}",
                                bufs=3)
                s4 = fpool.tile([P, B_LOCAL], f32, tag="sq", name=f"s4_{ic}")
                T4 = fpool.tile([P, B_LOCAL], f32, tag="Tn", name=f"T4_{ic}",
                                bufs=3)
                u5 = fpool.tile([P, B_LOCAL], f32, tag="u", name=f"u5_{ic}")
                s6 = fpool.tile([P, B_LOCAL], f32, tag="sq", name=f"s6_{ic}")
                u7 = fpool.tile([P, B_LOCAL], f32, tag="u", name=f"u7_{ic}")
                s8 = fpool.tile([P, B_LOCAL], f32, tag="sq", name=f"s8_{ic}")

                for sl in slices:
                    # T1 = tanh(x) (no clip: recurrence stable for |t|<=1;
                    # deviation from the reference's 0.999 clip is ~1e-6)
                    nc.scalar.activation(t[:, sl], xt_t[:, sl], AF.Tanh)
                    nc.vector.tensor_copy(pA[:, 0, sl], t[:, sl])

                    # T2 = 2 t^2 - 1
                    nc.scalar.square(s2[:, sl], t[:, sl])
                    nc.vector.tensor_scalar(T2[:, sl], s2[:, sl], 2.0, -1.0,
                                            ALU.mult, ALU.add)
                    nc.scalar.copy(pA[:, 1, sl], T2[:, sl])

                    # T3 = t * (2 T2 - 1)
                    nc.vector.tensor_scalar(V3[:, sl], T2[:, sl], 2.0, -1.0,
                                            ALU.mult, ALU.add)
                    nc.vector.tensor_mul(T3[:, sl], t[:, sl], V3[:, sl])
                    nc.scalar.copy(pB[:, ic % 2, sl], T3[:, sl])

                    # T4 = 2 T2^2 - 1
                    nc.scalar.square(s4[:, sl], T2[:, sl])
                    nc.vector.tensor_scalar(T4[:, sl], s4[:, sl], 2.0, -1.0,
                                            ALU.mult, ALU.add)
                    nc.scalar.copy(bb[:, 0, sl], T4[:, sl])

                    # T5 = 2 T2 T3 - T1, T7 = 2 T3 T4 - T1 (fp32 products so
                    # the bf16 basis keeps full bf16 accuracy)
                    nc.vector.tensor_mul(u5[:, sl], T2[:, sl], T3[:, sl])
                    nc.vector.scalar_tensor_tensor(bb[:, 1, sl], u5[:, sl],
                                                   2.0, t[:, sl],
                                                   ALU.mult, ALU.subtract)

                    # T6 = 2 T3^2 - 1, T8 = 2 T4^2 - 1
                    nc.scalar.square(s6[:, sl], T3[:, sl])
                    nc.vector.tensor_scalar(bb[:, 2, sl], s6[:, sl], 2.0, -1.0,
                                            ALU.mult, ALU.add)

                    nc.vector.tensor_mul(u7[:, sl], T3[:, sl], T4[:, sl])
                    nc.vector.scalar_tensor_tensor(bb[:, 3, sl], u7[:, sl],
                                                   2.0, t[:, sl],
                                                   ALU.mult, ALU.subtract)

                    nc.scalar.square(s8[:, sl], T4[:, sl])
                    nc.vector.tensor_scalar(bb[:, 4, sl], s8[:, sl], 2.0, -1.0,
                                            ALU.mult, ALU.add)

            # bias is only consumed at the end of each pass; load it late so
            # it doesn't delay the xt/wt streams.
            bias_t = cpool.tile([P, O_DIM], f32, name="bias_t")
            nc.sync.dma_start(out=bias_t, in_=bias_d[:, :])

            # ---- contraction passes ----
            psums = [ppool.tile([P, ON], f32, tag=f"ps{b}", name=f"ps{b}")
                     for b in range(B_LOCAL // P)]
            # pass 0: o-half 0, all 8 batch banks (overlaps basis production)
            # pass 1a/1b: o-half 1 split in two bank halves, so the first
            # half's drains + stores overlap the second half's matmuls.
            passes = [(0, 0, 8), (1, 0, 4), (1, 4, 8)]

            def get_st16(oh, j, pi):
                key = (oh, j) if pi == 0 else (oh, j, pi)
                if key not in st16:
                    wt = wpool.tile([P, 4, ON], bf16, tag="wt16",
                                    name=f"wt16_{pi}_{j}")
                    nc.gpsimd.dma_start(out=wt, in_=w16_d[oh, j])
                    st16[key] = wt
                return st16[key]

            def get_st8(oh, j, pi):
                key = (oh, j) if pi == 0 else (oh, j, pi)
                if key not in st8:
                    wt = w8pool.tile([P, 4, 2, ON], f8, tag="wt8",
                                     name=f"wt8_{pi}_{j}")
                    nc.gpsimd.dma_start(out=wt, in_=w8_d[oh, j])
                    st8[key] = wt
                return st8[key]

            for pi, (oh, blo, bhi) in enumerate(passes):
                n16 = 0
                n8 = 0
                for ci, (kind, idx) in enumerate(order):
                    st = (ci == 0)
                    sp = (ci == nchunk - 1)
                    if kind == "b":
                        wt = get_st16(oh, n16 // 4, pi)
                        c = n16 % 4
                        n16 += 1
                        ic, di = divmod(idx, NB16)
                        bt = b16[ic]
                        for b in range(blo, bhi):
                            nc.tensor.matmul(psums[b],
                                             bt[:, di, b * P:(b + 1) * P],
                                             wt[:, c, :],
                                             start=st, stop=sp)
                    else:
                        wt8 = get_st8(oh, n8 // 4, pi)
                        c = n8 % 4
                        n8 += 1
                        pt = pairs[idx]
                        for b in range(blo, bhi):
                            nc.tensor.matmul(psums[b],
                                             pt[:, :, b * P:(b + 1) * P],
                                             wt8[:, c, :, :],
                                             start=st, stop=sp, perf_mode=DR)

                for b in range(blo, bhi):
                    ot = opool.tile([P, ON], f32, tag="ot", name=f"ot_{pi}_{b}")
                    bias_sl = bias_t[:, oh * ON:(oh + 1) * ON]
                    if pi == 0 and b < 4:
                        # banks 0-3 gate pass 1a: drain via ACT copy (with the
                        # 2^-14 weight descale) so the start=True matmuls
                        # aren't stuck behind the serial DVE chain; add bias
                        # in place later (overlaps the next pass).
                        nc.scalar.activation(ot, psums[b], AF.Copy,
                                             scale=WSINV)
                        nc.vector.tensor_add(ot, ot, bias_sl)
                        nc.sync.dma_start(
                            out=y_d[b * P:(b + 1) * P,
                                    oh * ON:(oh + 1) * ON],
                            in_=ot)
                    elif pi == len(passes) - 1:
                        # final pass: split banks across ACT and DVE so the
                        # tail drains on two engines in parallel, at half-
                        # granularity so stores overlap the other half's add
                        for hh in range(2):
                            hsl = slice(hh * (ON // 2), (hh + 1) * (ON // 2))
                            if b % 2 == 0:
                                nc.scalar.activation(ot[:, hsl],
                                                     psums[b][:, hsl],
                                                     AF.Copy, scale=WSINV)
                                nc.vector.tensor_add(ot[:, hsl], ot[:, hsl],
                                                     bias_sl[:, hsl])
                            else:
                                nc.vector.scalar_tensor_tensor(
                                    ot[:, hsl], psums[b][:, hsl], WSINV,
                                    bias_sl[:, hsl], ALU.mult, ALU.add)
                            # alternate store queues: 8 back-to-back issues
                            # (~700ns each) would otherwise serialize the tail
                            q = nc.sync if b % 2 == 0 else nc.gpsimd
                            q.dma_start(
                                out=y_d[b * P:(b + 1) * P,
                                        oh * ON + hh * (ON // 2):
                                        oh * ON + (hh + 1) * (ON // 2)],
                                in_=ot[:, hsl])
                    else:
                        nc.vector.scalar_tensor_tensor(
                            ot, psums[b], WSINV, bias_sl, ALU.mult, ALU.add)
                        nc.sync.dma_start(
                            out=y_d[b * P:(b + 1) * P,
                                    oh * ON:(oh + 1) * ON],
                            in_=ot)
    nc.compile()  # bacc legalization: splits multi-sem waits (TRN2 allows 1)
    return nc


def _get_nc():
    global _nc
    if _nc is None:
        _nc = _build_nc()
    return _nc


def _prep_inputs(x, cheby_coeffs):
    x = np.asarray(x, dtype=np.float32)
    C = np.asarray(cheby_coeffs, dtype=np.float32)
    bf16 = ml_dtypes.bfloat16
    e4 = ml_dtypes.float8_e4m3

    Wd = C[:, :, 1:] * np.float32(WSCALE)              # [I, O, 8], d index 0..7
    Wd = Wd.reshape(IC, P, OH, ON, 8)                  # [ic, p, oh, on, d]

    # bf16 chunks: degrees 4..8, k16 = ic*5 + (d-4); packed [oh,st,p,4,on]
    W16 = np.transpose(Wd[:, :, :, :, 3:8], (2, 0, 4, 1, 3))  # [oh,ic,dd,p,on]
    W16 = W16.reshape(OH, NK16, P, ON).reshape(OH, NST16, 4, P, ON)
    W16 = np.ascontiguousarray(np.transpose(W16, (0, 1, 3, 2, 4))).astype(bf16)

    # fp8 pairs: A[ic] slots (d=1, d=2); B[j] slots (d=3 @ ic=2j, 2j+1);
    # packed [oh, st, p, 4, 2, on]
    W8 = np.empty((OH, NP8, P, 2, ON), np.float32)
    for ic in range(IC):
        W8[:, ic, :, 0, :] = np.transpose(Wd[ic, :, :, :, 0], (1, 0, 2))
        W8[:, ic, :, 1, :] = np.transpose(Wd[ic, :, :, :, 1], (1, 0, 2))
    for j in range(IC // 2):
        W8[:, IC + j, :, 0, :] = np.transpose(Wd[2 * j, :, :, :, 2], (1, 0, 2))
        W8[:, IC + j, :, 1, :] = np.transpose(Wd[2 * j + 1, :, :, :, 2],
                                              (1, 0, 2))
    # reorder pairs into the schedule sequence used by _chunk_order()
    p8_seq = [idx for kind, idx in _chunk_order() if kind == "p"]
    W8 = W8[:, p8_seq]
    W8 = W8.reshape(OH, NST8, 4, P, 2, ON)
    W8 = np.ascontiguousarray(np.transpose(W8, (0, 1, 3, 2, 4, 5))).astype(e4)

    bias = C[:, :, 0].sum(axis=0, dtype=np.float64).astype(np.float32)
    bias_rep = np.ascontiguousarray(np.broadcast_to(bias, (P, O_DIM)))

    in_maps = []
    for c in range(N_CORES):
        xt = np.ascontiguousarray(x[c * B_LOCAL:(c + 1) * B_LOCAL, :].T)
        in_maps.append({"xt": xt, "w16": W16, "w8": W8, "bias": bias_rep})
    return in_maps


def kernel(x, cheby_coeffs):
    global last_results
    nc = _get_nc()
    in_maps = _prep_inputs(x, cheby_coeffs)
    last_results = run_bass_kernel_spmd(nc, in_maps,
                                        core_ids=list(range(N_CORES)))
    y = np.concatenate([r["y"] for r in last_results.results], axis=0)
    return y
